# revision 20
# baseline (speedup 1.0000x reference)
"""Deformable patch embedding kernel for Trainium2 (Bass/Tile), 8-core data parallel.

v4 architecture (per core, 8 images; 12 chunks of 3 ho-rows, 128 partitions =
(b, hh, wo_l)), software-pipelined one chunk ahead:
  phase A (chunk N+1, emitted before phase B of chunk N):
    strip + transposed-im2col DMAs; offset conv on PE (cij on partitions),
    bias via K=1 matmul, psum->sbuf on ACT; ramp/tent tap weights on ACT;
    strip column-difference tensor D on DVE.
  phase B (chunk N):
    x-axis bilinear via the telescoped ramp identity (exact for |d| <= 2):
      V(d) = I[c0] + sum_{x=0..3} clamp(d+2-x, 0, 1) * D_x
    on DVE (one add per a-row on GPSIMD); y-axis 5-tap tent on GPSIMD
    (last chunk: on DVE to cut the tail); projection: PE transpose +
    ACT psum->sbuf, PE matmul + bias, bf16 out DMA. Host converts to f32.
"""

import os
import sys

for _p in ("/opt/trn_rl_repo", "/root/.axon_site/_ro/trn_rl_repo"):
    if os.path.isdir(_p) and _p not in sys.path:
        sys.path.insert(0, _p)

import numpy as np
import ml_dtypes

import concourse.bass as bass
import concourse.bacc as bacc
import concourse.mybir as mybir
import concourse.tile as tile
from concourse.alu_op_type import AluOpType as ALU

F32 = mybir.dt.float32
BF16 = mybir.dt.bfloat16

KS = 16          # patch/kernel size
CIN = 3
EMB = 768
CIJ = CIN * KS * KS          # 768 contraction size
KC = CIJ // 128              # 6 contraction chunks
OFFC = 2 * KS * KS           # 512 offset-conv out channels
TAPS = (-2, -1, 0, 1, 2)
DW = 19                      # D-tensor columns


class Cfg:
    def __init__(self, BL, HO):
        self.BL = BL                  # images per core
        self.HO = HO                  # patches per side
        self.H = HO * KS              # image side
        self.HOH = HO // 2            # ho per hh-half
        self.NPASS = 3
        assert HO % self.NPASS == 0
        self.WOPP = HO // self.NPASS  # wo columns per pass
        self.P = BL * self.WOPP * 2   # partitions used
        assert self.P == 128
        self.NCH = (3 if self.HOH % 3 == 0
                    else 2 if self.HOH % 2 == 0 else 1)
        self.NCHUNK = self.HOH // self.NCH
        self.RR = 16 * self.NCH + 4   # strip rows per chunk
        self.XS = 20                  # strip cols
        self.PQ = HO * HO             # positions per image
        self.NS = self.NCH * KS * KS  # samples per partition per chunk (768)
        self.MM = self.NCH * 128      # ximt columns per kc


def build_program(cfg: Cfg):
    """SPMD Bass program. Per-core inputs:
      strips [NP, NCK, BL, 2, WOPP, CIN, RR, XS]  bf16  col0 = 16*wo-2
      ximt   [NP, NCK, KC, 128, MM]               bf16  cij-major patch pixels
      woff   [769, 512]   bf16  rows cij, row 768 = bias
      pw     [769, 768]   bf16  rows cij, row 768 = bias
      ident  [128, 128]   bf16
    Output:
      out    [BL, PQ, 768] bf16   pos = ho*HO + wo
    """
    BL, HO, HOH, P = cfg.BL, cfg.HO, cfg.HOH, cfg.P
    WOPP = cfg.WOPP
    NCH, NCK, RR, XS, PQ, NS = (
        cfg.NCH, cfg.NCHUNK, cfg.RR, cfg.XS, cfg.PQ, cfg.NS)
    NP_, MM = cfg.NPASS, cfg.MM
    CNS = CIN * NS               # 2304
    NRW = KS * NCH               # 48 merged (h, i) rows

    nc = bacc.Bacc("TRN2", target_bir_lowering=False, debug=False)

    strips = nc.dram_tensor(
        "strips", [NP_, NCK, BL, 2, WOPP, CIN, RR, XS], BF16,
        kind="ExternalInput").ap()
    ximt = nc.dram_tensor("ximt", [NP_, NCK, KC, 128, MM], BF16,
                          kind="ExternalInput").ap()
    woff = nc.dram_tensor("woff", [CIJ + 1, OFFC], BF16,
                          kind="ExternalInput").ap()
    pw = nc.dram_tensor("pw", [CIJ + 1, EMB], BF16, kind="ExternalInput").ap()
    ident = nc.dram_tensor("ident", [128, 128], BF16, kind="ExternalInput").ap()
    out = nc.dram_tensor("out", [BL, PQ, EMB], BF16, kind="ExternalOutput").ap()

    with tile.TileContext(nc) as tc:
        import contextlib
        ctx = contextlib.ExitStack()
        with ctx:
            const = ctx.enter_context(tc.tile_pool(name="const", bufs=1))
            stripp = ctx.enter_context(tc.tile_pool(name="stripp", bufs=3))
            ximtp = ctx.enter_context(tc.tile_pool(name="ximtp", bufs=2))
            offsp = ctx.enter_context(tc.tile_pool(name="offsp", bufs=3))
            dp = ctx.enter_context(tc.tile_pool(name="dp", bufs=3))
            wtp = ctx.enter_context(tc.tile_pool(name="wtp", bufs=3))
            xcp = ctx.enter_context(tc.tile_pool(name="xcp", bufs=6))
            tmpxp = ctx.enter_context(tc.tile_pool(name="tmpxp", bufs=3))
            tmpyp = ctx.enter_context(tc.tile_pool(name="tmpyp", bufs=2))
            smp = ctx.enter_context(tc.tile_pool(name="smp", bufs=2))
            stp = ctx.enter_context(tc.tile_pool(name="stp", bufs=2))
            outp = ctx.enter_context(tc.tile_pool(name="outp", bufs=2))
            ps_off = ctx.enter_context(
                tc.tile_pool(name="ps_off", bufs=2, space="PSUM"))
            ps_t = ctx.enter_context(
                tc.tile_pool(name="ps_t", bufs=3, space="PSUM"))
            ps_o = ctx.enter_context(
                tc.tile_pool(name="ps_o", bufs=3, space="PSUM"))

            # ---- constants ----
            woff_sb = const.tile([128, KC * OFFC], BF16, tag="woff_sb")
            for k in range(KC):
                nc.sync.dma_start(woff_sb[:, k * OFFC:(k + 1) * OFFC],
                                  woff[k * 128:(k + 1) * 128, :])
            wob_sb = const.tile([1, OFFC], BF16, tag="wob_sb")
            nc.sync.dma_start(wob_sb[:], woff[CIJ:CIJ + 1, :])
            pw_sb = const.tile([128, KC * EMB], BF16, tag="pw_sb")
            for k in range(KC):
                nc.sync.dma_start(pw_sb[:, k * EMB:(k + 1) * EMB],
                                  pw[k * 128:(k + 1) * 128, :])
            pwb_sb = const.tile([1, EMB], BF16, tag="pwb_sb")
            nc.sync.dma_start(pwb_sb[:], pw[CIJ:CIJ + 1, :])
            id_sb = const.tile([128, 128], BF16, tag="id_sb")
            nc.sync.dma_start(id_sb[:], ident[:])
            ones_p = const.tile([1, 128], BF16, tag="ones_p")
            nc.vector.memset(ones_p[:], 1.0)
            tapb = {}
            for t in TAPS:
                bt_ = const.tile([128, 1], F32, tag=f"tapb{t}")
                nc.vector.memset(bt_[:], float(-t))
                tapb[t] = bt_
            rampb = {}
            for x in range(4):
                rb_ = const.tile([128, 1], F32, tag=f"rampb{x}")
                nc.vector.memset(rb_[:], float(x - 1))
                rampb[x] = rb_

            def phase_a(p, ch):
                """Chunk inputs + offset conv + tap weights + D. Returns state."""
                st_e = stripp.tile([P, CIN * RR * XS], BF16, tag="st_e")
                nc.sync.dma_start(
                    st_e[:],
                    strips[p, ch].rearrange("b h w c r x -> (b h w) (c r x)"))
                xt = ximtp.tile([128, KC * MM], BF16, tag="xt")
                xsrc = bass.AP(
                    ximt.tensor,
                    ximt.offset + (p * NCK + ch) * KC * 128 * MM,
                    [[MM, 128], [128 * MM, KC], [1, MM]])
                nc.scalar.dma_start(
                    xt[:].rearrange("q (k m) -> q k m", k=KC, m=MM), xsrc)

                offs = offsp.tile([P, NCH * OFFC], BF16, tag="offs")
                for h in range(NCH):
                    pso = ps_off.tile([128, OFFC], F32, tag="pso")
                    for kc in range(KC):
                        nc.tensor.matmul(
                            pso[:],
                            xt[:, kc * MM + h * 128:kc * MM + (h + 1) * 128],
                            woff_sb[:, kc * OFFC:(kc + 1) * OFFC],
                            start=(kc == 0), stop=False)
                    nc.tensor.matmul(
                        pso[:], ones_p[:], wob_sb[:], start=False, stop=True)
                    nc.scalar.copy(offs[:, h * OFFC:(h + 1) * OFFC], pso[:])

                # offs element (h, o) with o = 2*(16i+j) + comp
                def dview(comp):
                    return bass.AP(
                        offs[:].tensor, offs[:].offset + comp,
                        [offs[:].ap[0],
                         [OFFC, NCH], [2 * KS, KS], [2, KS]])

                # x ramp weights: cl[x] = clamp(dx + 2 - x, 0, 1)
                cl = {}
                for x in range(4):
                    r1 = tmpxp.tile([P, NS], F32, tag="u")
                    r1v = r1[:].rearrange("p (h i j) -> p h i j",
                                          h=NCH, i=KS, j=KS)
                    nc.scalar.activation(
                        r1v, dview(1), mybir.ActivationFunctionType.Relu,
                        bias=rampb[x][:P, :], scale=-1.0)
                    w = wtp.tile([P, NS], BF16, tag=f"cl{x}")
                    nc.scalar.activation(
                        w[:], r1[:], mybir.ActivationFunctionType.Relu,
                        bias=1.0, scale=-1.0)
                    cl[x] = w

                # y tent weights: w = relu(1 - |dy - t|)
                at = {}
                for t in TAPS:
                    u = tmpxp.tile([P, NS], F32, tag="u")
                    uv = u[:].rearrange("p (h i j) -> p h i j",
                                        h=NCH, i=KS, j=KS)
                    nc.scalar.activation(
                        uv, dview(0), mybir.ActivationFunctionType.Abs,
                        bias=tapb[t][:P, :], scale=1.0)
                    w = wtp.tile([P, NS], BF16, tag=f"wa{t}")
                    nc.scalar.activation(
                        w[:], u[:], mybir.ActivationFunctionType.Relu,
                        bias=1.0, scale=-1.0)
                    at[t] = w

                # D = column diffs of the strip (DVE)
                D = dp.tile([P, CIN * RR * DW], BF16, tag="D")
                Ddst = bass.AP(
                    D[:].tensor, D[:].offset,
                    [D[:].ap[0], [RR * DW, CIN], [DW, RR], [1, DW]])
                s_hi = bass.AP(
                    st_e[:].tensor, st_e[:].offset + 1,
                    [st_e[:].ap[0], [RR * XS, CIN], [XS, RR], [1, DW]])
                s_lo = bass.AP(
                    st_e[:].tensor, st_e[:].offset,
                    [st_e[:].ap[0], [RR * XS, CIN], [XS, RR], [1, DW]])
                nc.vector.tensor_tensor(Ddst, s_hi, s_lo, ALU.subtract)
                return dict(st_e=st_e, cl=cl, at=at, D=D)

            def phase_b(S, p, ch, last):
                st_e, cl, at, D = S["st_e"], S["cl"], S["at"], S["D"]

                def wview(w):
                    return bass.AP(
                        w[:].tensor, w[:].offset,
                        [w[:].ap[0], [0, CIN], [KS, NRW], [1, KS]])

                def dwin(a, x):
                    return bass.AP(
                        D[:].tensor, D[:].offset + (a + 2) * DW + x,
                        [D[:].ap[0], [RR * DW, CIN], [DW, NRW], [1, KS]])

                def iwin(a):
                    return bass.AP(
                        st_e[:].tensor, st_e[:].offset + (a + 2) * XS,
                        [st_e[:].ap[0], [RR * XS, CIN], [XS, NRW], [1, KS]])

                # x-stage ramp
                xc = {}
                for a in TAPS:
                    xca = xcp.tile([P, CNS], BF16, tag="xc")
                    xc[a] = xca
                    dstv = bass.AP(
                        xca[:].tensor, xca[:].offset,
                        [xca[:].ap[0], [NS, CIN], [KS, NRW], [1, KS]])
                    tmp = tmpxp.tile([P, CNS], BF16, tag="tmpx")
                    tv = bass.AP(
                        tmp[:].tensor, tmp[:].offset,
                        [tmp[:].ap[0], [NS, CIN], [KS, NRW], [1, KS]])
                    nc.vector.tensor_tensor(
                        tv, wview(cl[0]), dwin(a, 0), ALU.mult)
                    # acc = tmp + I[c0]; offload one add to gpsimd
                    eng = nc.gpsimd if (a == -2 and not last) else nc.vector
                    eng.tensor_tensor(dstv, tv, iwin(a), ALU.add)
                    for x in range(1, 4):
                        tmp = tmpxp.tile([P, CNS], BF16, tag="tmpx")
                        tv = bass.AP(
                            tmp[:].tensor, tmp[:].offset,
                            [tmp[:].ap[0], [NS, CIN], [KS, NRW], [1, KS]])
                        nc.vector.tensor_tensor(
                            tv, wview(cl[x]), dwin(a, x), ALU.mult)
                        nc.vector.tensor_tensor(
                            xca[:], xca[:], tmp[:], ALU.add)

                # y-stage tent; s_t layout per partition: (h, c, s)
                s_t = stp.tile([P, NCH * CIJ], BF16, tag="s_t")
                stv = bass.AP(
                    s_t[:].tensor, s_t[:].offset,
                    [s_t[:].ap[0],
                     [CIJ, NCH], [KS * KS, CIN], [1, KS * KS]])
                engy = nc.vector if last else nc.gpsimd
                for ai, a in enumerate(TAPS):
                    awv = bass.AP(
                        at[a][:].tensor, at[a][:].offset,
                        [at[a][:].ap[0],
                         [KS * KS, NCH], [0, CIN], [1, KS * KS]])
                    xcv = bass.AP(
                        xc[a][:].tensor, xc[a][:].offset,
                        [xc[a][:].ap[0],
                         [KS * KS, NCH], [NS, CIN], [1, KS * KS]])
                    if ai == 0:
                        engy.tensor_tensor(stv, awv, xcv, ALU.mult)
                    else:
                        tmp = tmpyp.tile([P, CNS], BF16, tag="tmpy")
                        tv = bass.AP(
                            tmp[:].tensor, tmp[:].offset,
                            [tmp[:].ap[0],
                             [CIJ, NCH], [KS * KS, CIN], [1, KS * KS]])
                        engy.tensor_tensor(tv, awv, xcv, ALU.mult)
                        engy.tensor_tensor(s_t[:], s_t[:], tmp[:], ALU.add)

                # projection
                outsb = outp.tile([P, NCH * EMB], BF16, tag="outsb")
                for ho_c in range(NCH):
                    sm = smp.tile([128, KC * P], BF16, tag="sm")
                    for kc in range(KC):
                        pst = ps_t.tile([128, P], BF16, tag="pst")
                        nc.tensor.transpose(
                            pst[:],
                            s_t[:, ho_c * CIJ + kc * 128:
                                ho_c * CIJ + (kc + 1) * 128],
                            id_sb[:P, :P])
                        nc.scalar.copy(sm[:, kc * P:(kc + 1) * P], pst[:])
                    for n in range(2):
                        psn = ps_o.tile([P, EMB // 2], F32, tag="psn")
                        for kc in range(KC):
                            nc.tensor.matmul(
                                psn[:], sm[:, kc * P:(kc + 1) * P],
                                pw_sb[:, kc * EMB + n * (EMB // 2):
                                      kc * EMB + (n + 1) * (EMB // 2)],
                                start=(kc == 0), stop=False)
                        nc.tensor.matmul(
                            psn[:], ones_p[:],
                            pwb_sb[:, n * (EMB // 2):(n + 1) * (EMB // 2)],
                            start=False, stop=True)
                        nc.scalar.copy(
                            outsb[:, ho_c * EMB + n * (EMB // 2):
                                  ho_c * EMB + (n + 1) * (EMB // 2)],
                            psn[:])

                return outsb

            def emit_out(outsb, p, ch):
                for b in range(BL):
                    for hh in range(2):
                        p0 = b * 2 * WOPP + hh * WOPP
                        dst_ap = bass.AP(
                            out.tensor,
                            out.offset + (b * PQ
                                          + (hh * HOH + NCH * ch) * HO
                                          + p * WOPP) * EMB,
                            [[EMB, WOPP], [HO * EMB, NCH], [1, EMB]])
                        nc.sync.dma_start(
                            dst_ap,
                            outsb[p0:p0 + WOPP, :].rearrange(
                                "w (h e) -> w h e", h=NCH, e=EMB))

            # software pipeline: phase A two chunks ahead; out DMAs one behind
            chunks = [(p, ch) for p in range(NP_) for ch in range(NCK)]
            Sq = [phase_a(*chunks[0]), phase_a(*chunks[1])]
            pend = None
            for i, (p, ch) in enumerate(chunks):
                if i + 2 < len(chunks):
                    Sq.append(phase_a(*chunks[i + 2]))
                if pend is not None:
                    emit_out(*pend)
                ob = phase_b(Sq.pop(0), p, ch, last=(i == len(chunks) - 1))
                pend = (ob, p, ch)
            emit_out(*pend)
    nc.compile()
    return nc


def prep_core_inputs(pv, woff_np, pw_np, cfg: Cfg):
    """pv: [BL, 3, H, H] f32 for this core. Returns the in_map dict."""
    BL, HO, HOH, XS = cfg.BL, cfg.HO, cfg.HOH, cfg.XS
    H = cfg.H
    NP_, NCK, NCH, RR, WOPP = cfg.NPASS, cfg.NCHUNK, cfg.NCH, cfg.RR, cfg.WOPP
    # padded image: rows/cols [-2, H+2)
    pad = np.zeros((BL, CIN, H + 5, H + 5), np.float32)
    pad[:, :, 2:2 + H, 2:2 + H] = pv
    sb = pad.strides
    s = np.lib.stride_tricks.as_strided(
        pad,
        shape=(NP_, NCK, BL, 2, WOPP, CIN, RR, XS),
        strides=(16 * WOPP * sb[3], 16 * NCH * sb[2], sb[0],
                 16 * HOH * sb[2], 16 * sb[3], sb[1], sb[2], sb[3]),
    )
    strips = np.ascontiguousarray(s).astype(ml_dtypes.bfloat16)
    # ximt[p, ch, kc, q, m]: q = c*256 + i*16 + j; m = h*128 + (b*16+hh*8+wo)
    x = pv.reshape(BL, CIN, 2, NCK, NCH, KS, NP_, WOPP, KS)
    #              b    c   hh  ch   h   i   p    wo   j
    x = x.transpose(6, 3, 1, 5, 8, 4, 0, 2, 7)   # p ch c i j h b hh wo
    x = np.ascontiguousarray(x).reshape(NP_, NCK, CIJ, NCH * 128)
    ximt = x.reshape(NP_, NCK, KC, 128, NCH * 128).astype(ml_dtypes.bfloat16)
    return {
        "strips": strips,
        "ximt": ximt,
        "woff": woff_np,
        "pw": pw_np,
        "ident": np.eye(128, dtype=ml_dtypes.bfloat16),
    }


def prep_weights(offset_w, offset_b, proj_w, proj_b):
    woff_np = np.concatenate(
        [offset_w.transpose(1, 2, 3, 0).reshape(CIJ, OFFC),
         offset_b.reshape(1, OFFC)], axis=0).astype(ml_dtypes.bfloat16)
    pw_np = np.concatenate(
        [proj_w.transpose(1, 2, 3, 0).reshape(CIJ, EMB),
         proj_b.reshape(1, EMB)], axis=0).astype(ml_dtypes.bfloat16)
    return woff_np, pw_np


_CACHE = {}


def kernel(pixel_values, offset_w, offset_b, proj_w, proj_b):
    from concourse.bass_utils import run_bass_kernel_spmd
    B = pixel_values.shape[0]
    n_cores = 8
    BL = B // n_cores
    HO = pixel_values.shape[2] // KS
    cfg = Cfg(BL, HO)
    key = (BL, HO)
    if key not in _CACHE:
        _CACHE[key] = build_program(cfg)
    nc = _CACHE[key]
    woff_np, pw_np = prep_weights(
        np.asarray(offset_w), np.asarray(offset_b),
        np.asarray(proj_w), np.asarray(proj_b))
    pv = np.asarray(pixel_values, dtype=np.float32).reshape(
        n_cores, BL, CIN, cfg.H, cfg.H)
    in_maps = [prep_core_inputs(pv[c], woff_np, pw_np, cfg)
               for c in range(n_cores)]
    res = run_bass_kernel_spmd(nc, in_maps, core_ids=list(range(n_cores)))
    outs = [res.results[c]["out"].reshape(BL, cfg.PQ, EMB).astype(np.float32)
            for c in range(n_cores)]
    return np.concatenate(outs, axis=0)
